# revision 12
# baseline (speedup 1.0000x reference)
"""Block-diagonal linear kernel for 8 TRN2 NeuronCores.

Problem: x [4096, 8192] fp32, blocks [64, 128, 128] fp32,
out[b, n*128+r] = sum_c x[b, n*128+c] * blocks[n, r, c].

Sharding: block-parallel (expert-style). Core k owns blocks 8k..8k+7, the
matching x column-slice x[:, 1024k:1024(k+1)] and output column-slice
out[:, 1024k:1024(k+1)]. Communication-free.

Layout: the PE contracts over the partition dim, so x must be presented
feature-major. fp32 has no DMA-transpose path on TRN2, so the host hands
each core xT = x[:, cols].T (contiguous row-slab of the host-transposed x)
and receives outT = out[:, cols].T back. On-device everything is then plain
contiguous streaming:
  per block i: load xT slab [128, 4096] (2 MiB, one DMA)
               8x matmul(psum[r=128, 512] = blockT_i.T @ xT_slab[:, j*512:])
               copy psum -> out slab (alternating DVE / ACT)
               store outT slab [128, 4096] (2 MiB, one DMA)

The kernel is DMA-bound: ~33 MiB of HBM traffic per core at ~358 GB/s
(~95 us floor); PE/DVE/ACT work fits underneath.
"""

import numpy as np

import concourse.mybir as mybir
import concourse.tile as tile
from concourse import bacc, bass_utils

N_CORES = 8
N_BLOCKS = 64
BLK = 128                      # block rows/cols
BATCH = 4096
D = N_BLOCKS * BLK             # 8192
BPC = N_BLOCKS // N_CORES      # 8 blocks per core
CLS = BPC * BLK                # 1024: column-slice width per core
NCHUNK = 512                   # matmul moving-dim (fp32 PSUM bank limit)
NB = BATCH // NCHUNK           # 8 batch chunks

_CACHE = {}

# Matmul-input dtype: float32r streams 1 row/cycle on the PE (vs 4 for
# float32) at slightly reduced multiply precision; bytes and numpy dtype
# are identical to float32.
MM_DT = "float32r"


def _emit_body(nc, xpool, opool, pspool, w_sb, xt, outt):
    """One full pass over the core's shard.

    Two 128-row slabs per DMA (4 MiB transfers). Loads issue from the SP
    HWDGE ring (nc.sync), stores from the ACT ring (nc.scalar) so the two
    streams don't serialize in one FIFO.
    """
    f32 = mybir.dt.float32
    mmdt = getattr(mybir.dt, MM_DT)
    xt2 = xt.rearrange("(g two p) b -> g p two b", two=2, p=BLK)    # [4,128,2,B]
    outt2 = outt.rearrange("(g two p) b -> g p two b", two=2, p=BLK)
    for g in range(BPC // 2):
        x_sb = xpool.tile([BLK, 2, BATCH], mmdt)
        nc.sync.dma_start(out=x_sb, in_=xt2[g])
        o_sb = opool.tile([BLK, 2, BATCH], f32)
        for t in range(2):
            i = 2 * g + t
            for j in range(NB):
                ps = pspool.tile([BLK, NCHUNK], f32)
                nc.tensor.matmul(
                    ps,
                    lhsT=w_sb[:, i, :],
                    rhs=x_sb[:, t, j * NCHUNK : (j + 1) * NCHUNK],
                    start=True,
                    stop=True,
                )
                # split the 16 MiB of PSUM->SBUF copies across DVE and ACT
                if j % 2 == 0:
                    nc.vector.tensor_copy(
                        out=o_sb[:, t, j * NCHUNK : (j + 1) * NCHUNK], in_=ps
                    )
                else:
                    nc.scalar.copy(o_sb[:, t, j * NCHUNK : (j + 1) * NCHUNK], ps)
        nc.scalar.dma_start(out=outt2[g], in_=o_sb)


def _build_bass(iters: int = 1, loop_iters: int = 0, loop_unroll: int = 4):
    """One SPMD program; every core runs it on its own shard.

    iters > 1 (python-unrolled) or loop_iters > 0 (device For_i around
    loop_unroll python-unrolled passes) repeat the body with identical I/O —
    used only for timing via the slope method (axon dispatch overhead,
    ~80 ms, dominates any single wall-clock call).
    """
    nc = bacc.Bacc("TRN2", debug=False, num_devices=N_CORES, target_bir_lowering=False)
    f32 = mybir.dt.float32
    mmdt = getattr(mybir.dt, MM_DT)
    xt = nc.dram_tensor("xt", [CLS, BATCH], mmdt, kind="ExternalInput").ap()
    wt = nc.dram_tensor("wt", [BPC, BLK, BLK], mmdt, kind="ExternalInput").ap()
    outt = nc.dram_tensor("outt", [CLS, BATCH], f32, kind="ExternalOutput").ap()

    with tile.TileContext(nc) as tc:
        with (
            tc.tile_pool(name="w", bufs=1) as wpool,
            tc.tile_pool(name="xin", bufs=2) as xpool,
            tc.tile_pool(name="xout", bufs=2) as opool,
            tc.tile_pool(name="ps", bufs=8, space="PSUM") as pspool,
        ):
            # blockT weights, resident for the whole kernel: [c=128, i, r]
            w_sb = wpool.tile([BLK, BPC, BLK], mmdt)
            for i in range(BPC):
                nc.sync.dma_start(out=w_sb[:, i, :], in_=wt[i])

            if loop_iters > 0:
                with tc.For_i(0, loop_iters, 1):
                    for _ in range(loop_unroll):
                        _emit_body(nc, xpool, opool, pspool, w_sb, xt, outt)
            else:
                for _ in range(iters):
                    _emit_body(nc, xpool, opool, pspool, w_sb, xt, outt)
    nc.compile()
    return nc


def _get_bass():
    if "nc" not in _CACHE:
        _CACHE["nc"] = _build_bass()
    return _CACHE["nc"]


def _make_in_maps(x: np.ndarray, blocks: np.ndarray):
    xT = np.ascontiguousarray(x.T)  # [8192, 4096]
    blocksT = np.ascontiguousarray(blocks.transpose(0, 2, 1))  # [n, c, r]
    in_maps = []
    for k in range(N_CORES):
        in_maps.append(
            {
                "xt": xT[CLS * k : CLS * (k + 1)],
                "wt": blocksT[BPC * k : BPC * (k + 1)],
            }
        )
    return in_maps


def _gather(results):
    out = np.empty((BATCH, D), dtype=np.float32)
    for k in range(N_CORES):
        out[:, CLS * k : CLS * (k + 1)] = results[k]["outt"].T
    return out


def kernel(x: np.ndarray, blocks: np.ndarray) -> np.ndarray:
    nc = _get_bass()
    in_maps = _make_in_maps(np.asarray(x, np.float32), np.asarray(blocks, np.float32))
    res = bass_utils.run_bass_kernel_spmd(nc, in_maps, core_ids=list(range(N_CORES)))
    return _gather(res.results)


# revision 13
# speedup vs baseline: 1.0162x; 1.0162x over previous
"""Block-diagonal linear kernel for 8 TRN2 NeuronCores.

Problem: x [4096, 8192] fp32, blocks [64, 128, 128] fp32,
out[b, n*128+r] = sum_c x[b, n*128+c] * blocks[n, r, c].

Sharding: block-parallel (expert-style). Core k owns blocks 8k..8k+7, the
matching x column-slice x[:, 1024k:1024(k+1)] and output column-slice
out[:, 1024k:1024(k+1)]. Communication-free.

Layout: the PE contracts over the partition dim, so x must be presented
feature-major. fp32 has no DMA-transpose path on TRN2, so the host hands
each core xT = x[:, cols].T (contiguous row-slab of the host-transposed x)
and receives outT = out[:, cols].T back. On-device everything is then plain
contiguous streaming:
  per block i: load xT slab [128, 4096] (2 MiB, one DMA)
               8x matmul(psum[r=128, 512] = blockT_i.T @ xT_slab[:, j*512:])
               copy psum -> out slab (alternating DVE / ACT)
               store outT slab [128, 4096] (2 MiB, one DMA)

The kernel is DMA-bound: ~33 MiB of HBM traffic per core at ~358 GB/s
(~95 us floor); PE/DVE/ACT work fits underneath.
"""

import numpy as np

import concourse.mybir as mybir
import concourse.tile as tile
from concourse import bacc, bass_utils

N_CORES = 8
N_BLOCKS = 64
BLK = 128                      # block rows/cols
BATCH = 4096
D = N_BLOCKS * BLK             # 8192
BPC = N_BLOCKS // N_CORES      # 8 blocks per core
CLS = BPC * BLK                # 1024: column-slice width per core
NCHUNK = 512                   # matmul moving-dim (fp32 PSUM bank limit)
NB = BATCH // NCHUNK           # 8 batch chunks

_CACHE = {}

# Matmul-input dtype: float32r streams 1 row/cycle on the PE (vs 4 for
# float32) at slightly reduced multiply precision; bytes and numpy dtype
# are identical to float32.
MM_DT = "float32r"


def _emit_body(nc, xpool, opool, pspool, w_sb, xt, outt):
    """One full pass over the core's shard.

    Two 128-row slabs per DMA (4 MiB transfers). Loads issue from the SP
    HWDGE ring (nc.sync), stores from the ACT ring (nc.scalar) so the two
    streams don't serialize in one FIFO.
    """
    f32 = mybir.dt.float32
    mmdt = getattr(mybir.dt, MM_DT)
    xt2 = xt.rearrange("(g two p) b -> g p two b", two=2, p=BLK)    # [4,128,2,B]
    outt2 = outt.rearrange("(g two p) b -> g p two b", two=2, p=BLK)
    for g in range(BPC // 2):
        x_sb = xpool.tile([BLK, 2, BATCH], mmdt)
        nc.sync.dma_start(out=x_sb, in_=xt2[g])
        o_sb = opool.tile([BLK, 2, BATCH], f32)
        for t in range(2):
            i = 2 * g + t
            for j in range(NB):
                ps = pspool.tile([BLK, NCHUNK], f32)
                nc.tensor.matmul(
                    ps,
                    lhsT=w_sb[:, i, :],
                    rhs=x_sb[:, t, j * NCHUNK : (j + 1) * NCHUNK],
                    start=True,
                    stop=True,
                )
                # split the 16 MiB of PSUM->SBUF copies across DVE and ACT
                if j % 2 == 0:
                    nc.vector.tensor_copy(
                        out=o_sb[:, t, j * NCHUNK : (j + 1) * NCHUNK], in_=ps
                    )
                else:
                    nc.scalar.copy(o_sb[:, t, j * NCHUNK : (j + 1) * NCHUNK], ps)
        nc.scalar.dma_start(out=outt2[g], in_=o_sb)


def _build_bass(iters: int = 1, loop_iters: int = 0, loop_unroll: int = 4):
    """One SPMD program; every core runs it on its own shard.

    iters > 1 (python-unrolled) or loop_iters > 0 (device For_i around
    loop_unroll python-unrolled passes) repeat the body with identical I/O —
    used only for timing via the slope method (axon dispatch overhead,
    ~80 ms, dominates any single wall-clock call).
    """
    nc = bacc.Bacc("TRN2", debug=False, num_devices=N_CORES, target_bir_lowering=False)
    f32 = mybir.dt.float32
    mmdt = getattr(mybir.dt, MM_DT)
    xt = nc.dram_tensor("xt", [CLS, BATCH], mmdt, kind="ExternalInput").ap()
    wt = nc.dram_tensor("wt", [BPC, BLK, BLK], mmdt, kind="ExternalInput").ap()
    outt = nc.dram_tensor("outt", [CLS, BATCH], f32, kind="ExternalOutput").ap()

    with tile.TileContext(nc) as tc:
        with (
            tc.tile_pool(name="w", bufs=1) as wpool,
            tc.tile_pool(name="xin", bufs=2) as xpool,
            tc.tile_pool(name="xout", bufs=3) as opool,
            tc.tile_pool(name="ps", bufs=8, space="PSUM") as pspool,
        ):
            # blockT weights, resident for the whole kernel: [c=128, i, r]
            w_sb = wpool.tile([BLK, BPC, BLK], mmdt)
            for i in range(BPC):
                nc.sync.dma_start(out=w_sb[:, i, :], in_=wt[i])

            if loop_iters > 0:
                with tc.For_i(0, loop_iters, 1):
                    for _ in range(loop_unroll):
                        _emit_body(nc, xpool, opool, pspool, w_sb, xt, outt)
            else:
                for _ in range(iters):
                    _emit_body(nc, xpool, opool, pspool, w_sb, xt, outt)
    nc.compile()
    return nc


def _get_bass():
    if "nc" not in _CACHE:
        _CACHE["nc"] = _build_bass()
    return _CACHE["nc"]


def _make_in_maps(x: np.ndarray, blocks: np.ndarray):
    xT = np.ascontiguousarray(x.T)  # [8192, 4096]
    blocksT = np.ascontiguousarray(blocks.transpose(0, 2, 1))  # [n, c, r]
    in_maps = []
    for k in range(N_CORES):
        in_maps.append(
            {
                "xt": xT[CLS * k : CLS * (k + 1)],
                "wt": blocksT[BPC * k : BPC * (k + 1)],
            }
        )
    return in_maps


def _gather(results):
    out = np.empty((BATCH, D), dtype=np.float32)
    for k in range(N_CORES):
        out[:, CLS * k : CLS * (k + 1)] = results[k]["outt"].T
    return out


def kernel(x: np.ndarray, blocks: np.ndarray) -> np.ndarray:
    nc = _get_bass()
    in_maps = _make_in_maps(np.asarray(x, np.float32), np.asarray(blocks, np.float32))
    res = bass_utils.run_bass_kernel_spmd(nc, in_maps, core_ids=list(range(N_CORES)))
    return _gather(res.results)
